# revision 1
# baseline (speedup 1.0000x reference)
"""Trainium2 kernel for AutoPatchOverLapModel3D (3D patch overlap-add / fold).

Math: out[b,p,y0,y1,y2] = (1/CM[y0,y1,y2]) * sum_{j0,j1,j2}
        x[b, y0-j0, y1-j1, (y2-j2)%64, p, j0, j1, j2]
i.e. a stride-1 overlap-add of 5x5x5 patches; axes 0/1 zero-padded,
axis 2 circular; CM is the separable patch-count normalizer.

Strategy (8 NeuronCores, SPMD):
  - The patch index n factors as n = col*64 + i2 with col=(b*10+i0)*28+i1
    (560 columns of 64 circularly-coupled patches each). Shard 70
    columns per core: each core reads a contiguous 44.8 MB slice.
  - On device, fold the circular i2/j2 axis with TensorE matmuls:
    128 patches (2 columns) per group on the contraction axis, using a
    block-diagonal 0/1 shift-weight matrix per j2 tap (5 taps
    accumulated in PSUM).  out_col[y2, (p,j0,j1)] per column.
  - The tiny j0/j1 overlap-add across columns (5x5 shifted adds of a
    4.6 MB result) and the CM division run on the host.
"""

import numpy as np

B, X0, X1, X2, P = 2, 10, 28, 64, 20
PK = 5  # patch edge
Y0, Y1, Y2 = 14, 32, 64
NCOL = B * X0 * X1            # 560 (b,i0,i1) columns
NCORES = 8
COLS_PER_CORE = NCOL // NCORES  # 70
PATCH_VEC = P * PK * PK * PK    # 2500
FREE = P * PK * PK              # 500 = (p, j0, j1)
GROUPS = COLS_PER_CORE * X2 // 128  # 35 groups of 128 patches (2 cols)
FRAMES = 5                      # half-plane frames per core (14 cols each)
GROUPS_PER_FRAME = 7
YF = 18                         # y1 span within a half-plane: 13 + 4 + 1

_CACHE = {}


def _shift_weights():
    # w[k, j2*128 + m]: k = u*64 + i2, m = u*64 + y2 ;  1.0 iff same u
    # and y2 == (i2 + j2 - 2) % 64 (the reference's circular axis keeps
    # patch centers at their own index: tap j2 lands at offset j2-2).
    # Block-diagonal over the 2 columns that share a matmul group.
    w = np.zeros((128, 5, 128), np.float32)
    i2 = np.arange(64)
    for j2 in range(5):
        y2 = (i2 + j2 - 2) % 64
        for u in range(2):
            w[u * 64 + i2, j2, u * 64 + y2] = 1.0
    return w.reshape(128, 5 * 128)


def _kernel_body(tc, xs, w, out):
    import concourse.mybir as mybir

    nc = tc.nc
    f32 = mybir.dt.float32
    f32r = xs.dtype  # float32r on HW (fast fp32 matmul path), f32 in sim
    with (
        tc.tile_pool(name="wpool", bufs=1) as wpool,
        tc.tile_pool(name="xpool", bufs=8) as xpool,
        tc.tile_pool(name="accpool", bufs=3) as accpool,
        tc.tile_pool(name="pspool", bufs=6, space="PSUM") as pspool,
    ):
        wt = wpool.tile([128, 5 * 128], f32r)
        nc.sync.dma_start(out=wt[:, :], in_=w[:, :])
        # 5 half-plane frames of 7 groups (14 columns) each; frame
        # boundaries are half-plane aligned on every core (70 % 14 == 0),
        # keeping the program SPMD-uniform.
        for h in range(FRAMES):
            acc = accpool.tile([128, 100 * YF], f32)
            nc.gpsimd.memset(acc[:, :], 0.0)
            av = acc[:, :].rearrange("a (f y) -> a y f", y=YF)
            for q in range(GROUPS_PER_FRAME):
                g = h * GROUPS_PER_FRAME + q
                xt = xpool.tile([128, PATCH_VEC], f32r)
                nc.sync.dma_start(
                    out=xt[:, :], in_=xs[g * 128:(g + 1) * 128, :]
                )
                ps = pspool.tile([128, FREE], f32)
                xv = xt[:, :].rearrange("a (f j) -> a j f", j=5)
                for j2 in range(5):
                    nc.tensor.matmul(
                        ps[:, :],
                        wt[:, j2 * 128:(j2 + 1) * 128],
                        xv[:, j2, :],
                        start=(j2 == 0),
                        stop=(j2 == 4),
                    )
                # fold j1 on-device: column i1 = 2q+u lands at y1f = i1+j1.
                # One 3D-AP add per u-block covers all 5 j1 taps at once
                # (dst y1f window [2q+u, 2q+u+5) is stride-1, like j1).
                pv = ps[:, :].rearrange("a (f j) -> a j f", j=5)
                for u in range(2):
                    lo = 2 * q + u
                    dst = av[u * 64:(u + 1) * 64, lo:lo + 5, :]
                    nc.vector.tensor_add(
                        dst, dst, pv[u * 64:(u + 1) * 64, :, :]
                    )
            nc.gpsimd.dma_start(out=out[h, :, :], in_=acc[:, :])


def _build_nc():
    import concourse.bacc as bacc
    import concourse.mybir as mybir
    import concourse.tile as tile

    nc = bacc.Bacc(
        "TRN2",
        target_bir_lowering=False,
        debug=False,
        enable_asserts=True,
        num_devices=NCORES,
    )
    f32 = mybir.dt.float32
    xs = nc.declare_dram_parameter("xs", [COLS_PER_CORE * 64, PATCH_VEC], mybir.dt.float32r, isOutput=False)
    w = nc.declare_dram_parameter("w", [128, 5 * 128], mybir.dt.float32r, isOutput=False)
    out = nc.declare_dram_parameter("out", [FRAMES, 128, 100 * YF], f32, isOutput=True)

    with tile.TileContext(nc) as tc:
        _kernel_body(tc, xs, w, out)
    nc.compile()
    return nc


def _counting_matrix():
    c0 = np.zeros(Y0, np.float32)
    for i0 in range(X0):
        c0[i0:i0 + PK] += 1
    c1 = np.zeros(Y1, np.float32)
    for i1 in range(X1):
        c1[i1:i1 + PK] += 1
    return c0[:, None, None] * c1[None, :, None] * 5.0


def kernel(x: np.ndarray) -> np.ndarray:
    from concourse.bass_utils import run_bass_kernel_spmd

    if "nc" not in _CACHE:
        _CACHE["nc"] = _build_nc()
    nc = _CACHE["nc"]

    xf = np.ascontiguousarray(x, np.float32).reshape(NCOL * X2, PATCH_VEC)
    wnp = _shift_weights()
    rows = COLS_PER_CORE * X2
    in_maps = [
        {"xs": xf[c * rows:(c + 1) * rows], "w": wnp} for c in range(NCORES)
    ]
    res = run_bass_kernel_spmd(nc, in_maps, list(range(NCORES)))
    oc = np.stack([res.results[c]["out"] for c in range(NCORES)], axis=0)

    # host stitch: oc[c, h] holds half-plane H=5c+h partials
    # [(u, y2), (p, j0, y1f)]; place at y1 = 14*(H%2) + y1f, y0 = i0 + j0.
    ocr = oc.reshape(NCORES * FRAMES, 2, 64, P, PK, YF)     # H,u,y2,p,j0,y1f
    ocr = ocr.sum(1).transpose(0, 2, 3, 4, 1)               # H,p,j0,y1f,y2
    out = np.zeros((B, P, Y0, Y1, Y2), np.float32)
    for H in range(NCORES * FRAMES):
        gp, half = divmod(H, 2)
        b, i0 = divmod(gp, X0)
        y1lo = (X1 // 2) * half
        out[b, :, i0:i0 + PK, y1lo:y1lo + YF, :] += ocr[H]
    out /= _counting_matrix()
    return out



# revision 4
# speedup vs baseline: 1.7738x; 1.7738x over previous
"""Trainium2 kernel for AutoPatchOverLapModel3D (3D patch overlap-add / fold).

Math: out[b,p,y0,y1,y2] = (1/CM[y0,y1,y2]) * sum_{j0,j1,j2}
        x[b, y0-j0, y1-j1, (y2-j2)%64, p, j0, j1, j2]
i.e. a stride-1 overlap-add of 5x5x5 patches; axes 0/1 zero-padded,
axis 2 circular; CM is the separable patch-count normalizer.

Strategy (8 NeuronCores, SPMD), v2 — memory-roofline oriented:
  - Host casts x to bf16 (RNE; tolerance is 2e-2, bf16 costs ~2e-3) and
    permutes each 2500-vec patch to (j2, j0, j1, p) so every j2 tap is a
    contiguous 500-elem slice. HBM read per core: 22.4 MB instead of 44.8.
  - Shard 5 half-planes (70 columns = 4480 patch rows) per core.
  - Per 128-patch group (2 columns): fold the circular j2 axis with 5
    TensorE matmuls (block-diag 0/1 shift weights, bf16, PSUM f32).
  - Fold j1 AND j0 on-device with one 4D-AP tensor_add per column into a
    persistent accumulator acc[(u,y2), k, y1', p] (k = frame-pair-local
    y0, y1' = 36 = two 18-wide half-plane windows). u=0 adds run on DVE,
    u=1 on Pool — two independent chains on disjoint partition ranges.
  - k-planes are flushed to DRAM as soon as no later frame can touch
    them (after groups 13 / 27 / 34), shrinking the un-overlapped tail.
  - Host: sum u, place per-core (k, s) cells at (y0, y1) (core-parity
    mapping below), divide by the counting matrix.
"""

import numpy as np

B, X0, X1, X2, P = 2, 10, 28, 64, 20
PK = 5  # patch edge
Y0, Y1, Y2 = 14, 32, 64
NCORES = 8
NCOL = B * X0 * X1                   # 560 (b,i0,i1) columns
COLS_PER_CORE = NCOL // NCORES       # 70
ROWS_PER_CORE = COLS_PER_CORE * X2   # 4480
PATCH_VEC = P * PK * PK * PK         # 2500
FREE = PK * PK * P                   # 500 per j2 tap, laid out (j0, j1, p)
GROUPS = ROWS_PER_CORE // 128        # 35 groups of 128 patches (2 cols)
GROUPS_PER_FRAME = 7                 # 14 columns = one half-plane frame
FRAMES = 5
KSPAN = 7                            # frame-local y0 span: 3 i0 values + 4
Y1SPAN = 36                          # two 18-wide half-plane y1 windows
ACC_FREE = KSPAN * Y1SPAN * P        # 5040
K0_END = KSPAN * 0 + 720             # free offset where k=0 plane ends
K1_END = 2 * 720

_CACHE = {}


def _shift_weights():
    # w[k, j2*128 + m]: k = u*64 + i2, m = u*64 + y2 ;  1.0 iff same u
    # and y2 == (i2 + j2 - 2) % 64 (circular axis keeps patch centers at
    # their own index: tap j2 lands at offset j2-2). Block-diagonal over
    # the 2 columns sharing a matmul group.
    w = np.zeros((128, 5, 128), np.float32)
    i2 = np.arange(64)
    for j2 in range(5):
        y2 = (i2 + j2 - 2) % 64
        for u in range(2):
            w[u * 64 + i2, j2, u * 64 + y2] = 1.0
    return w.reshape(128, 5 * 128)


def _kernel_body(tc, xs, w, out):
    import concourse.mybir as mybir

    nc = tc.nc
    f32 = mybir.dt.float32
    with (
        tc.tile_pool(name="wpool", bufs=1) as wpool,
        tc.tile_pool(name="xpool", bufs=8) as xpool,
        tc.tile_pool(name="accpool", bufs=1) as accpool,
        tc.tile_pool(name="pspool", bufs=6, space="PSUM") as pspool,
    ):
        wt = wpool.tile([128, 5 * 128], xs.dtype)
        nc.sync.dma_start(out=wt[:, :], in_=w[:, :])
        acc = accpool.tile([128, ACC_FREE], f32)
        nc.gpsimd.memset(acc[:, :], 0.0)
        av = acc[:, :].rearrange("a (k y p) -> a k y p", k=KSPAN, y=Y1SPAN)
        for g in range(GROUPS):
            h, q = divmod(g, GROUPS_PER_FRAME)
            k0, s = divmod(h, 2)
            xt = xpool.tile([128, PATCH_VEC], xs.dtype)
            nc.sync.dma_start(out=xt[:, :], in_=xs[g * 128:(g + 1) * 128, :])
            ps = pspool.tile([128, FREE], f32)
            for j2 in range(5):
                nc.tensor.matmul(
                    ps[:, :],
                    wt[:, j2 * 128:(j2 + 1) * 128],
                    xt[:, j2 * FREE:(j2 + 1) * FREE],
                    start=(j2 == 0),
                    stop=(j2 == 4),
                )
            # ps free layout (j0, j1, p) -> dst windows k0+j0, y1b+j1: one
            # 4D-AP accumulate per group. The u=1 column's extra +1 y1
            # offset is absorbed into the layout (its cells are stored
            # one slot early; the host shifts them back), so a single
            # 128-partition DVE add folds both columns at once.
            pv = ps[:, :].rearrange("a (j0 j1 p) -> a j0 j1 p", j0=PK, j1=PK)
            y1b = 18 * s + 2 * q
            dst = av[:, k0:k0 + 5, y1b:y1b + 5, :]
            nc.vector.tensor_add(dst, dst, pv[:, :, :, :])
            # flush k-planes no later frame can write (h>=2 writes k>=1,
            # h>=4 writes k>=2): overlap output DMA with remaining input.
            if g == 2 * GROUPS_PER_FRAME - 1:
                nc.scalar.dma_start(out=out[:, 0:720], in_=acc[:, 0:720])
            elif g == 4 * GROUPS_PER_FRAME - 1:
                nc.scalar.dma_start(out=out[:, 720:1440], in_=acc[:, 720:1440])
        nc.scalar.dma_start(out=out[:, 1440:ACC_FREE], in_=acc[:, 1440:ACC_FREE])


def _build_nc():
    import concourse.bacc as bacc
    import concourse.mybir as mybir
    import concourse.tile as tile

    nc = bacc.Bacc(
        "TRN2",
        target_bir_lowering=False,
        debug=False,
        enable_asserts=True,
        num_devices=NCORES,
    )
    f32 = mybir.dt.float32
    bf16 = mybir.dt.bfloat16
    xs = nc.declare_dram_parameter(
        "xs", [ROWS_PER_CORE, PATCH_VEC], bf16, isOutput=False
    )
    w = nc.declare_dram_parameter("w", [128, 5 * 128], bf16, isOutput=False)
    out = nc.declare_dram_parameter("out", [128, ACC_FREE], f32, isOutput=True)

    with tile.TileContext(nc) as tc:
        _kernel_body(tc, xs, w, out)
    nc.compile()
    return nc


def _counting_matrix():
    c0 = np.zeros(Y0, np.float32)
    for i0 in range(X0):
        c0[i0:i0 + PK] += 1
    c1 = np.zeros(Y1, np.float32)
    for i1 in range(X1):
        c1[i1:i1 + PK] += 1
    return c0[:, None, None] * c1[None, :, None] * 5.0


def _make_in_maps(x):
    import ml_dtypes

    # bf16 RNE cast first (contiguous, fast), then patch-dim permute
    # (p, j0, j1, j2) -> (j2, j0, j1, p) so each j2 tap is a contiguous
    # 500-elem slice whose (j0, j1, p) order matches the accumulator.
    xb = x.reshape(NCOL * X2, P, PK, PK, PK).astype(ml_dtypes.bfloat16)
    xb = np.ascontiguousarray(xb.transpose(0, 4, 2, 3, 1)).reshape(
        NCOL * X2, PATCH_VEC
    )
    wnp = _shift_weights().astype(ml_dtypes.bfloat16)
    return [
        {"xs": xb[c * ROWS_PER_CORE:(c + 1) * ROWS_PER_CORE], "w": wnp}
        for c in range(NCORES)
    ]


def _stitch(oc):
    # oc: [c, 128, 5040] -> [c, u, y2, k, s, y1f, p].
    # Device frame h wrote (k0=h//2, s=h%2). True (i0rel, half) per core
    # parity: even cores (h//2, h%2); odd cores ((h+1)//2, (h+1)%2) — so
    # cell (k, s) is (y0 = i0a + k, half = s) on even cores and
    # (y0 = i0a + k + s, half = 1-s) on odd cores. The u=1 column's
    # cells are stored one y1 slot early (see kernel body): shift by +u.
    ocr = oc.reshape(NCORES, 2, 64, KSPAN, 2, 18, P)
    out = np.zeros((B, P, Y0, Y1, Y2), np.float32)
    for c in range(NCORES):
        g0 = (5 * c) // 2
        b, i0a = divmod(g0, X0)
        odd = c % 2
        for s in range(2):
            half = (1 - s) if odd else s
            dy0 = i0a + (s if odd else 0)
            kmax = min(KSPAN, Y0 - dy0)  # trailing cells beyond Y0 are 0
            for u in range(2):
                wid = 18 - u  # u=1's last stored slot is never written
                blk = ocr[c, u, :, :kmax, s, :wid, :]    # [y2, k, y1f, p]
                y1lo = 14 * half + u
                out[b, :, dy0:dy0 + kmax, y1lo:y1lo + wid, :] += (
                    blk.transpose(3, 1, 2, 0)
                )
    return out / _counting_matrix()


def kernel(x: np.ndarray) -> np.ndarray:
    from concourse.bass_utils import run_bass_kernel_spmd

    if "nc" not in _CACHE:
        _CACHE["nc"] = _build_nc()
    nc = _CACHE["nc"]
    in_maps = _make_in_maps(x)
    res = run_bass_kernel_spmd(nc, in_maps, list(range(NCORES)))
    oc = np.stack([res.results[c]["out"] for c in range(NCORES)], axis=0)
    return _stitch(oc.astype(np.float32))
